# revision 26
# baseline (speedup 1.0000x reference)
"""Exact top-k (k=32) attention on 8 Trainium2 NeuronCores.

Head-parallel: 16 (batch, head) pairs sharded 2-per-core. Per head, per
128-query L-tile:
  1. [PE]     forward scores F[q, s] via 2-pass bf16-split matmul
              (hi*hi + partial lo*lo; hi*lo + lo*hi), accurate to ~1e-5.
  2. [ScalarE] E = exp(temp * F) fp32, straight from PSUM (the top-32 of E
              equals the top-32 of F; normalization by Z makes any
              threshold shift unnecessary, so no bias is needed).
  3. [DVE]    exact top-32 of E via a two-level scheme: 16 chunk-max8's
              (top-8 of each 128-wide chunk -> 128 candidates), then 4
              rounds of max8/match_replace on the candidates.  The 32nd
              largest candidate t equals the true 32nd largest value
              unless some chunk holds >= 9 of the row's top-32; such rows
              are flagged (a chunk's 8th-largest candidate >= threshold)
              and recomputed exactly on the host (~1-2% of rows).
  4. [GpSimd] A = (E > t_minus) * E in one pass (bf16 out), with
              t_minus = t*(1-2^-23) - 1e-37 strictly inside (E_33, E_32).
  5. [DMA]    A[q, s] -> A^T[s, q] chunks via the XBAR dma transpose
              (idle DMA engines; no PE/PSUM involvement).
  6. [PE]     AV: A^T chunk stationary, V (with ones column) moving ->
              PSUM [q, 65]; col 64 is Z = sum of selected weights.
  7. [DVE]    out = AV * (1/Z); flag count stored in output col 64.
Engines are balanced: DVE does only selection (+tiny normalize), ScalarE
only exp, GpSimd the masked apply, DMA the transpose, PE the matmuls.
"""

import numpy as np
import ml_dtypes

N, L, S, H, E, D = 2, 2048, 2048, 8, 64, 64
TOPK = 32
TEMP = 1.0 / np.sqrt(E)
HEADS_PER_CORE = 2
N_CORES = 8
LT = 16          # L tiles of 128
CHUNKS = 16      # s chunks of 128
NEG = -1e30
NLO = 63         # e-rows of the lo*lo partial correction in pass A

_bf16 = ml_dtypes.bfloat16


def _build_bass():
    import concourse.mybir as mybir
    from concourse import bacc
    from concourse.tile import TileContext

    f32 = mybir.dt.float32
    bf16 = mybir.dt.bfloat16
    Alu = mybir.AluOpType

    nc = bacc.Bacc()
    HPC = HEADS_PER_CORE

    qa_d = nc.declare_dram_parameter("qa", [HPC, 128, L], bf16, isOutput=False)
    ka_d = nc.declare_dram_parameter("ka", [HPC, 128, S], bf16, isOutput=False)
    qbc_d = nc.declare_dram_parameter("qbc", [HPC, 128, L], bf16, isOutput=False)
    kbc_d = nc.declare_dram_parameter("kbc", [HPC, 128, S], bf16, isOutput=False)
    va_d = nc.declare_dram_parameter("va", [HPC, CHUNKS, 128, D + 1], bf16,
                                     isOutput=False)
    out_d = nc.declare_dram_parameter("out", [HPC, L, D + 1], bf16,
                                      isOutput=True)
    # per-tile loose-chunk evidence: 16 chunk 8th-largest values + t32
    aux_d = nc.declare_dram_parameter("aux", [HPC, LT, 128, CHUNKS + 1], f32,
                                      isOutput=True)

    from contextlib import ExitStack
    with TileContext(nc) as tc, ExitStack() as ctx:
        inpool = ctx.enter_context(tc.tile_pool(name="inputs", bufs=1))
        consts = ctx.enter_context(tc.tile_pool(name="consts", bufs=1))
        epool = ctx.enter_context(tc.tile_pool(name="ebuf", bufs=7))
        apool = ctx.enter_context(tc.tile_pool(name="abuf", bufs=2))
        atpool = ctx.enter_context(tc.tile_pool(name="atbuf", bufs=2))
        spool = ctx.enter_context(tc.tile_pool(name="small", bufs=4))
        opool = ctx.enter_context(tc.tile_pool(name="outbuf", bufs=3))
        ps_f = ctx.enter_context(tc.tile_pool(name="ps_fwd", bufs=3,
                                              space="PSUM"))
        ps_av = ctx.enter_context(tc.tile_pool(name="ps_av", bufs=2,
                                               space="PSUM"))

        zcol = consts.tile([128, 1], f32)
        nc.vector.memset(zcol, 0.0)

        qa = []
        ka = []
        qbc = []
        kbc = []
        va = []
        for hh in range(HPC):
            qa.append(inpool.tile([128, L], bf16, tag=f"qa{hh}",
                                  name=f"qa{hh}"))
            ka.append(inpool.tile([128, S], bf16, tag=f"ka{hh}",
                                  name=f"ka{hh}"))
            qbc.append(inpool.tile([128, L], bf16, tag=f"qbc{hh}",
                                   name=f"qbc{hh}"))
            kbc.append(inpool.tile([128, S], bf16, tag=f"kbc{hh}",
                                   name=f"kbc{hh}"))
            va.append(inpool.tile([128, CHUNKS, D + 1], bf16, tag=f"va{hh}",
                                  name=f"va{hh}"))
        # load order: first fwd tile's operands land first
        nc.sync.dma_start(qa[0], qa_d[0])
        nc.sync.dma_start(ka[0][:, 0:1024], ka_d[0][:, 0:1024])
        nc.sync.dma_start(qbc[0], qbc_d[0])
        nc.sync.dma_start(kbc[0][:, 0:1024], kbc_d[0][:, 0:1024])
        nc.sync.dma_start(ka[0][:, 1024:S], ka_d[0][:, 1024:S])
        nc.sync.dma_start(kbc[0][:, 1024:S], kbc_d[0][:, 1024:S])
        nc.sync.dma_start(va[0], va_d[0].rearrange("c p d -> p c d"))
        nc.sync.dma_start(qa[1], qa_d[1])
        nc.sync.dma_start(ka[1], ka_d[1])
        nc.sync.dma_start(qbc[1], qbc_d[1])
        nc.sync.dma_start(kbc[1], kbc_d[1])
        nc.sync.dma_start(va[1], va_d[1].rearrange("c p d -> p c d"))

        e_tiles = {}

        def stage_fwd(hh, lt):
            """Forward scores -> E = exp(temp * F) in fp32 SBUF."""
            ls = slice(lt * 128, (lt + 1) * 128)
            e_sb = epool.tile([128, S], f32, tag="E", name="E")
            e_tiles[(hh, lt)] = e_sb
            for half in range(2):
                pf = ps_f.tile([128, 1024], f32, tag="fwd", name="fwd")
                for q4 in range(2):
                    sblk = slice(half * 1024 + q4 * 512,
                                 half * 1024 + (q4 + 1) * 512)
                    pblk = slice(q4 * 512, (q4 + 1) * 512)
                    nc.tensor.matmul(pf[:, pblk], qa[hh][:, ls],
                                     ka[hh][:, sblk], start=True, stop=False)
                    nc.tensor.matmul(pf[:, pblk], qbc[hh][:, ls],
                                     kbc[hh][:, sblk], start=False, stop=True)
                nc.scalar.activation(
                    e_sb[:, half * 1024:(half + 1) * 1024], pf,
                    mybir.ActivationFunctionType.Exp, scale=float(TEMP))

        tmp_tiles = {}
        tmn_tiles = {}
        r_tiles = {}

        def stage_sel(hh, lt):
            e_sb = e_tiles[(hh, lt)]
            # two-level exact top-32 of E; round 0's match_replace writes a
            # scratch copy so `cand` stays intact for the flag-evidence DMA
            cand = spool.tile([128, CHUNKS, 8], f32, tag="cand", name="cand")
            for c in range(CHUNKS):
                nc.vector.max(out=cand[:, c, :],
                              in_=e_sb[:, c * 128:(c + 1) * 128])
            cand2 = spool.tile([128, CHUNKS, 8], f32, tag="cand2",
                               name="cand2")
            m32 = spool.tile([128, 32], f32, tag="m32", name="m32")
            for r in range(4):
                nc.vector.max(out=m32[:, 8 * r:8 * r + 8],
                              in_=cand if r == 0 else cand2)
                if r < 3:
                    nc.vector.match_replace(
                        out=cand2, in_to_replace=m32[:, 8 * r:8 * r + 8],
                        in_values=cand if r == 0 else cand2, imm_value=-1.0)
            # chunk 8th-largest values + t32 straight to DRAM for host flags
            nc.sync.dma_start(aux_d[hh, lt, :, 0:CHUNKS], cand[:, :, 7])
            nc.sync.dma_start(aux_d[hh, lt, :, CHUNKS:CHUNKS + 1],
                              m32[:, 31:32])
            # tmn = -tau (just below the 32nd-largest E) on ScalarE; tmp = -tmn
            # on GpSimd (Copy supports out = in*scale + bias with float bias)
            tmp = spool.tile([128, 1], f32, tag="tmp", name="tmp")
            tmn = spool.tile([128, 1], f32, tag="tmn", name="tmn")
            c23 = float(1.0 - 2.0 ** -23)
            nc.scalar.activation(tmn, m32[:, 31:32],
                                 mybir.ActivationFunctionType.Copy,
                                 scale=-c23, bias=1e-37)
            nc.gpsimd.tensor_tensor(out=tmp, in0=zcol, in1=tmn,
                                    op=Alu.subtract)
            tmp_tiles[(hh, lt)] = tmp
            tmn_tiles[(hh, lt)] = tmn

        def stage_relu(hh, lt):
            # R = relu(E - tau) on ScalarE; emitted early so it is never
            # queued behind future exps when the DVE needs it
            e_sb = e_tiles.pop((hh, lt))
            tmn = tmn_tiles.pop((hh, lt))
            r_sb = apool.tile([128, S], bf16, tag="R", name="R")
            nc.scalar.activation(r_sb, e_sb,
                                 mybir.ActivationFunctionType.Relu,
                                 bias=tmn, scale=1.0)
            r_tiles[(hh, lt)] = r_sb

        def stage_apply(hh, lt):
            # tau*M01 on DVE (bf16 4x), A = R + tau*M01 on GpSimd
            r_sb = r_tiles.pop((hh, lt))
            tmp = tmp_tiles.pop((hh, lt))
            m01t = apool.tile([128, S], bf16, tag="M01t", name="M01t")
            nc.vector.tensor_scalar(
                out=m01t, in0=r_sb, scalar1=0.0, scalar2=tmp,
                op0=Alu.is_gt, op1=Alu.mult)
            b_sb = apool.tile([128, S], bf16, tag="B", name="B")
            nc.gpsimd.tensor_tensor(out=b_sb, in0=r_sb, in1=m01t, op=Alu.add)
            # transpose A -> [s-part, chunk, q-slice of quarter buffer]
            qi = lt % QT
            at_q = at_cur[0]
            sl = slice(qi * 128, (qi + 1) * 128)
            nc.sync.dma_start_transpose(at_q[:, :, sl], b_sb)

        def stage_av_mm(hh, qt):
            """AV for a 4-tile quarter: V stationary, 512 q-cols moving."""
            at_q = at_cur[0]
            avp = ps_av.tile([128, 512], f32, tag="av", name="av")
            for c in range(CHUNKS):
                nc.tensor.matmul(avp[0:D + 1, :], va[hh][:, c, :],
                                 at_q[:, c, :],
                                 start=(c == 0), stop=(c == CHUNKS - 1))
            return avp

        def stage_av_out(hh, qt, avp):
            # [d, q] -> bf16 staging (rows D+1..79 zeroed) -> xbar transpose;
            # emitted one iteration after the matmuls so the scalar copy
            # never parks the scalar queue on a PE wait
            st = opool.tile([80, 512], bf16, tag="avstage", name="avstage")
            nc.gpsimd.memset(st[D:80, :], 0.0)
            nc.scalar.copy(out=st[0:D + 1, :], in_=avp[0:D + 1, :])
            o_t = opool.tile([128, 4, 80], bf16, tag="ot", name="ot")
            nc.sync.dma_start_transpose(o_t, st)
            qs = qt * 512
            nc.sync.dma_start(
                out_d[hh, qs:qs + 512, :].rearrange("(c p) j -> p c j", c=4),
                o_t[:, :, 0:D + 1])

        tiles = [(hh, lt) for hh in range(HPC) for lt in range(LT)]
        NT = len(tiles)
        QT = 4    # tiles per AV quarter
        LOOK = 4
        at_cur = [None]
        av_pending = [None]
        for i in range(NT + LOOK + 2):
            k = i - LOOK - 1   # apply stage (skewed one behind selection)
            j = i - LOOK       # selection stage
            if 0 <= k < NT:
                stage_relu(*tiles[k])
            if av_pending[0] is not None:
                stage_av_out(*av_pending[0])
                av_pending[0] = None
            if i < NT:
                stage_fwd(*tiles[i])
            if 0 <= j < NT:
                stage_sel(*tiles[j])
            if 0 <= k < NT:
                hh, lt = tiles[k]
                if lt % QT == 0:
                    at_cur[0] = atpool.tile([128, CHUNKS, QT * 128], bf16,
                                            tag="AT", name="AT")
                stage_apply(hh, lt)
                if lt % QT == QT - 1:
                    avp = stage_av_mm(hh, lt // QT)
                    av_pending[0] = (hh, lt // QT, avp)

    nc.compile()
    return nc


_NC_CACHE = None


def _get_nc():
    global _NC_CACHE
    if _NC_CACHE is None:
        _NC_CACHE = _build_bass()
    return _NC_CACHE


def _split_hi_lo(x):
    hi = x.astype(_bf16)
    lo = (x.astype(np.float32) - hi.astype(np.float32)).astype(_bf16)
    return hi, lo


def _host_fix_rows(out, fix, queries, keys, values, key_lengths):
    """Exact fp32 recompute of flagged rows, batched per (n, h)."""
    by_nh = {}
    for (n, lq, h) in fix:
        by_nh.setdefault((n, h), []).append(lq)
    for (n, h), rows in by_nh.items():
        rows = np.asarray(rows)
        Q = np.asarray(queries[n, rows, h, :], np.float32)   # [m, E]
        K = np.asarray(keys[n, :, h, :], np.float32)         # [S, E]
        V = np.asarray(values[n, :, h, :], np.float32)       # [S, D]
        kl = int(key_lengths[n])
        sc = Q @ K.T                                         # [m, S]
        sc[:, kl:] = -np.inf
        idx = np.argpartition(-sc, TOPK - 1, axis=1)[:, :TOPK]
        ssel = np.take_along_axis(sc, idx, axis=1)
        w = np.exp(TEMP * (ssel - ssel.max(axis=1, keepdims=True)))
        w /= w.sum(axis=1, keepdims=True)
        out[n, rows, h, :] = np.einsum("ms,msd->md", w, V[idx],
                                       optimize=True)


# Interleave keys over s so valid (unmasked) keys spread evenly across the
# 16 selection chunks; attention output is permutation-invariant over keys.
_PERM = np.arange(S).reshape(128, CHUNKS).T.reshape(-1)


def _prep_core(core, queries, keys, values, key_lengths_i):
    pairs = [(core // 4, (core % 4) * 2), (core // 4, (core % 4) * 2 + 1)]
    qa = np.zeros((HEADS_PER_CORE, 128, L), _bf16)
    ka = np.zeros((HEADS_PER_CORE, 128, S), _bf16)
    qbc = np.zeros((HEADS_PER_CORE, 128, L), _bf16)
    kbc = np.zeros((HEADS_PER_CORE, 128, S), _bf16)
    va = np.zeros((HEADS_PER_CORE, CHUNKS, 128, D + 1), _bf16)
    for i, (n, h) in enumerate(pairs):
        Q = queries[n, :, h, :]           # [L, E]
        K = keys[n, _PERM, h, :]          # [S, E], interleaved
        V = values[n, _PERM, h, :]        # [S, D], interleaved
        qh, ql = _split_hi_lo(Q)
        kh, kl_ = _split_hi_lo(K)
        mask = np.where(_PERM < int(key_lengths_i[n]), 0.0, NEG
                        ).astype(np.float32)
        qa[i, 0:E, :] = qh.T
        qa[i, E, :] = 1.0
        qa[i, E + 1:E + 1 + NLO, :] = ql.T[0:NLO]
        ka[i, 0:E, :] = kh.T
        ka[i, E, :] = mask.astype(_bf16)
        ka[i, E + 1:E + 1 + NLO, :] = kl_.T[0:NLO]
        qbc[i, 0:E, :] = qh.T
        qbc[i, E:2 * E, :] = ql.T
        kbc[i, 0:E, :] = kl_.T
        kbc[i, E:2 * E, :] = kh.T
        va[i, :, :, 0:D] = V.astype(_bf16).reshape(CHUNKS, 128, D)
        va[i, :, :, D] = 1.0
    return pairs, {"qa": qa, "ka": ka, "qbc": qbc, "kbc": kbc, "va": va}


def kernel(queries, keys, values, key_lengths):
    from concourse.bass_utils import run_bass_kernel_spmd

    queries = np.asarray(queries, np.float32)
    keys = np.asarray(keys, np.float32)
    values = np.asarray(values, np.float32)
    key_lengths_i = np.asarray(key_lengths).astype(np.int64)

    in_maps = []
    head_map = []
    for core in range(N_CORES):
        pairs, im = _prep_core(core, queries, keys, values, key_lengths_i)
        head_map.append(pairs)
        in_maps.append(im)

    nc = _get_nc()
    res = run_bass_kernel_spmd(nc, in_maps, list(range(N_CORES)))

    out = np.zeros((N, L, H, D), np.float32)
    fix_rows = []
    c23 = np.float32(1.0 - 2.0 ** -23)
    for core in range(N_CORES):
        o = np.asarray(res.results[core]["out"], np.float32
                       ).reshape(HEADS_PER_CORE, L, D + 1)
        aux = res.results[core]["aux"].reshape(HEADS_PER_CORE, L, CHUNKS + 1)
        for i, (n, h) in enumerate(head_map[core]):
            out[n, :, h, :] = o[i, :, 0:D] / o[i, :, D:D + 1]
            tau = aux[i, :, CHUNKS] * c23 - np.float32(1e-37)
            bad = np.nonzero((aux[i, :, 0:CHUNKS] > tau[:, None]).any(1))[0]
            for lq in bad:
                fix_rows.append((n, int(lq), h))
    if fix_rows:
        _host_fix_rows(out, fix_rows, queries, keys, values, key_lengths_i)
    return out
